# revision 20
# baseline (speedup 1.0000x reference)
"""Trainium2 Bass kernel for nn_Attention_40037685133427.

FiLM-conditioned LayerNorm + 16-head self-attention (B=2, N=2048, D=1024),
tensor-parallel over 8 NeuronCores: core c owns heads {2c, 2c+1}.

Per-core dataflow (everything transposed-native, [feature, token] layouts):
  - LN stats via PE ones-matmuls over bf16 x^T (cross-partition sums),
    rstd = exp(-0.5*ln(var+eps)) on ACT, per-token stats broadcast across
    partitions with Kc=1 f32r matmuls.
  - FiLM folded in as per-partition tensor_scalar (gamma', beta' columns).
  - QKV with column-sliced weights -> q^T,k^T,v^T [128, 4096] bf16.
  - V re-transposed to natural layout via PE transpose.
  - S^T = K Q^T with two heads row-tiled concurrently (Kc=64 each).
  - exp on ACT with the 1/sqrt(dh) scale folded in (no max subtraction:
    |S| < ~4 by construction).
  - attn@V col-tiled (h0 -> psum rows 0:64, h1 -> 64:128) + softmax
    denominators via ones-column matmuls into one PSUM tile (rows 0/32/64/96).
  - normalization fused into the PSUM->SBUF evacuation (TT multiply with a
    PE-broadcast reciprocal-denominator tile).
  - y^T = Wo_slice^T-layout matmul over the fused 128-wide head slice.
Host sums the 8 partial y^T outputs (row-split Wo => partial sums).
"""

import sys

sys.path.insert(0, "/opt/trn_rl_repo")

import numpy as np
import ml_dtypes

import concourse.bass as bass
from concourse import bacc
import concourse.tile as tile
from concourse import mybir
from concourse.bass_utils import run_bass_kernel_spmd
from concourse.masks import make_identity

f32 = mybir.dt.float32
f32r = mybir.dt.float32r
bf16 = mybir.dt.bfloat16
AF = mybir.ActivationFunctionType
ALU = mybir.AluOpType

B, N, DIM = 2, 2048, 1024
HEADS, DH = 16, 64
TOK = B * N            # 4096 tokens, batch-major
KT = DIM // 128        # 8 k-tiles over the model dim
NSL = TOK // 512       # 8 token slices of 512
JT = N // 128          # 16 key tiles per batch
COND = 1024
NCORES = 8


def build_program():
    nc = bacc.Bacc("TRN2", target_bir_lowering=False, debug=False)

    xT = nc.dram_tensor("xT", [DIM, TOK], f32, kind="ExternalInput").ap()
    ceT = nc.dram_tensor("ceT", [128, 2 * KT], f32, kind="ExternalInput").ap()
    gammaT = nc.dram_tensor("gammaT", [128, KT], f32, kind="ExternalInput").ap()
    condW = nc.dram_tensor("condW", [COND, 2 * DIM], f32, kind="ExternalInput").ap()
    condb = nc.dram_tensor("condb", [2, 2 * DIM], f32, kind="ExternalInput").ap()
    wqkv = nc.dram_tensor("wqkv", [DIM, 384], f32, kind="ExternalInput").ap()
    wo = nc.dram_tensor("wo", [128, DIM], f32, kind="ExternalInput").ap()
    ones2_in = nc.dram_tensor("ones2", [2, 128], bf16, kind="ExternalInput").ap()

    yT_out = nc.dram_tensor("yT", [DIM, TOK], bf16, kind="ExternalOutput").ap()

    # internal DRAM bounce buffers
    film_d = nc.dram_tensor("film_d", [2, 2, KT, 128], f32).ap()   # (b, scale/shift, kt, p)
    stats_d = nc.dram_tensor("stats_d", [2, TOK], f32).ap()        # (sum|sumsq, tok)
    um_d = nc.dram_tensor("um_d", [2, TOK], bf16).ap()             # (u|m, tok)
    den_d = nc.dram_tensor("den_d", [B, 4, 2, 512], f32).ap()      # (b, isl, h, x)
    r_d = nc.dram_tensor("r_d", [B, 4, 2, 512], bf16).ap()

    with tile.TileContext(nc) as tc:
        with (
            tc.tile_pool(name="const", bufs=1) as const,
            tc.tile_pool(name="persist", bufs=1) as persist,
            tc.tile_pool(name="work", bufs=3) as work,
            tc.tile_pool(name="ps", bufs=8, space="PSUM") as ps,
        ):
            def pst(shape=(128, 512), dtype=f32):
                return ps.tile(list(shape), dtype, tag="ps", name="pstile")

            # ---------------- constants / weights ----------------
            ident = const.tile([128, 128], bf16)
            make_identity(nc, ident[:])
            ones_col = const.tile([128, 1], bf16)
            nc.vector.memset(ones_col[:], 1.0)
            ones1 = const.tile([1, 128], bf16)
            nc.vector.memset(ones1[:], 1.0)
            ones2 = const.tile([2, 128], bf16)
            nc.sync.dma_start(ones2[:], ones2_in)

            wqkv_bf = []
            for kt in range(KT):
                wf = work.tile([128, 384], f32, tag="wf", bufs=2)
                nc.sync.dma_start(wf[:], wqkv[kt * 128:(kt + 1) * 128, :])
                wb = persist.tile([128, 384], bf16, tag="wqkv", bufs=KT)
                nc.any.tensor_copy(wb[:], wf[:])
                wqkv_bf.append(wb)
            wof = work.tile([128, DIM], f32, tag="wof", bufs=1)
            nc.sync.dma_start(wof[:], wo)
            wo_bf = persist.tile([128, DIM], bf16, tag="wo")
            nc.any.tensor_copy(wo_bf[:], wof[:])

            gam = const.tile([128, KT], f32)
            nc.sync.dma_start(gam[:], gammaT)
            cet = const.tile([128, 2 * KT], f32)
            nc.sync.dma_start(cet[:], ceT)

            # ---------------- FiLM conditioning ----------------
            sil = const.tile([128, 2 * KT], f32)
            # silu(x) = x / (1 + exp(-x)) -- built from Exp so only one ACT table set is used
            nc.scalar.activation(sil[:], cet[:], AF.Exp, scale=-1.0)
            nc.vector.tensor_scalar(sil[:], sil[:], 1.0, None, ALU.add)
            nc.vector.reciprocal(sil[:], sil[:])
            nc.vector.tensor_tensor(sil[:], sil[:], cet[:], op=ALU.mult)
            sil_bf = const.tile([128, 2 * KT], bf16)
            nc.vector.tensor_copy(sil_bf[:], sil[:])
            film_flat = film_d.rearrange("b s k p -> b (s k p)")
            for cs in range(4):
                pc = pst((2, 512))
                for kt in range(KT):
                    cw = work.tile([128, 512], f32, tag="cw", bufs=2)
                    nc.sync.dma_start(cw[:], condW[kt * 128:(kt + 1) * 128, cs * 512:(cs + 1) * 512])
                    cwb = work.tile([128, 512], bf16, tag="cwb", bufs=2)
                    nc.any.tensor_copy(cwb[:], cw[:])
                    nc.tensor.matmul(
                        pc[:],
                        sil_bf[:, 2 * kt:2 * kt + 2],
                        cwb[:],
                        start=(kt == 0), stop=(kt == KT - 1),
                    )
                sl = slice(cs * 512, (cs + 1) * 512)
                cbw = work.tile([2, 512], f32, tag="cbw", bufs=2)
                nc.sync.dma_start(cbw[:], condb[:, sl])
                csl = work.tile([2, 512], f32, tag="csl", bufs=2)
                nc.vector.tensor_tensor(csl[:], pc[:], cbw[:], op=ALU.add)
                nc.sync.dma_start(film_flat[:, sl], csl[:])
            gp = const.tile([128, 2 * KT], f32)   # gamma' columns, col = b*KT + kt
            bp = const.tile([128, 2 * KT], f32)   # beta'
            for b in range(B):
                sl = slice(b * KT, (b + 1) * KT)
                nc.sync.dma_start(gp[:, sl], film_d[b, 0].rearrange("k p -> p k"))
                nc.sync.dma_start(bp[:, sl], film_d[b, 1].rearrange("k p -> p k"))
            gpf = const.tile([128, 2 * KT], f32)
            nc.vector.tensor_scalar(gpf[:], gp[:], 1.0, None, ALU.add)
            for b in range(B):
                sl = slice(b * KT, (b + 1) * KT)
                nc.vector.tensor_tensor(gpf[:, sl], gpf[:, sl], gam[:], op=ALU.mult)

            # ---------------- load x^T, cast to bf16 ----------------
            x_bf = []
            for kt in range(KT):
                xb = persist.tile([128, TOK], bf16, tag="xbf", bufs=KT)
                for h in range(4):
                    xf = work.tile([128, 1024], f32, tag="xf", bufs=2)
                    nc.sync.dma_start(xf[:], xT[kt * 128:(kt + 1) * 128, h * 1024:(h + 1) * 1024])
                    nc.any.tensor_copy(xb[:, h * 1024:(h + 1) * 1024], xf[:])
                x_bf.append(xb)

            # ---------------- LN stats ----------------
            for isl in range(NSL):
                sl = slice(isl * 512, (isl + 1) * 512)
                psu = pst((1, 512))
                psq = pst((1, 512))
                for kt in range(KT):
                    xsq = work.tile([128, 512], bf16, tag="xsq", bufs=3)
                    nc.vector.tensor_tensor(xsq[:], x_bf[kt][:, sl], x_bf[kt][:, sl], op=ALU.mult)
                    nc.tensor.matmul(psu[:], ones_col[:], x_bf[kt][:, sl],
                                     start=(kt == 0), stop=(kt == KT - 1))
                    nc.tensor.matmul(psq[:], ones_col[:], xsq[:],
                                     start=(kt == 0), stop=(kt == KT - 1))
                surow = work.tile([1, 512], f32, tag="statrow", bufs=2)
                nc.any.tensor_copy(surow[:], psu[:])
                nc.sync.dma_start(stats_d[0:1, sl], surow[:])
                sqrow = work.tile([1, 512], f32, tag="statrow", bufs=2)
                nc.any.tensor_copy(sqrow[:], psq[:])
                nc.sync.dma_start(stats_d[1:2, sl], sqrow[:])

            NC32 = TOK // 128  # 32 columns when packed [128, 32]
            sc = work.tile([128, 2 * NC32], f32, tag="sc", bufs=1)
            nc.sync.dma_start(sc[:], stats_d.rearrange("s (c p) -> p (s c)", p=128))
            mean_t = work.tile([128, NC32], f32, tag="mean", bufs=1)
            var_t = work.tile([128, NC32], f32, tag="var", bufs=1)
            u_t = work.tile([128, NC32], f32, tag="ut", bufs=1)
            m_t = work.tile([128, NC32], f32, tag="mt", bufs=1)
            nc.vector.tensor_scalar(mean_t[:], sc[:, 0:NC32], 1.0 / DIM, None, ALU.mult)
            nc.vector.tensor_scalar(var_t[:], sc[:, NC32:], 1.0 / DIM, None, ALU.mult)
            msq = work.tile([128, NC32], f32, tag="msq", bufs=1)
            nc.vector.tensor_tensor(msq[:], mean_t[:], mean_t[:], op=ALU.mult)
            nc.vector.tensor_tensor(var_t[:], var_t[:], msq[:], op=ALU.subtract)
            # rstd = exp(-0.5 * ln(var + eps))
            eps_t = const.tile([128, 1], f32)
            nc.vector.memset(eps_t[:], 1e-5)
            nc.scalar.activation(var_t[:], var_t[:], AF.Ln, bias=eps_t[:])
            nc.scalar.activation(u_t[:], var_t[:], AF.Exp, scale=-0.5)
            nc.vector.tensor_tensor(m_t[:], mean_t[:], u_t[:], op=ALU.mult)
            ub_t = work.tile([128, NC32], bf16, tag="ubt", bufs=1)
            mb_t = work.tile([128, NC32], bf16, tag="mbt", bufs=1)
            nc.vector.tensor_copy(ub_t[:], u_t[:])
            nc.vector.tensor_copy(mb_t[:], m_t[:])
            nc.sync.dma_start(um_d.rearrange("s (c p) -> s p c", p=128)[0], ub_t[:])
            nc.sync.dma_start(um_d.rearrange("s (c p) -> s p c", p=128)[1], mb_t[:])
            # broadcast u/m across partitions via Kc=1 matmuls
            U_sb, M_sb = [], []
            for isl in range(NSL):
                sl = slice(isl * 512, (isl + 1) * 512)
                ur = work.tile([1, 512], bf16, tag="umrow", bufs=2)
                nc.sync.dma_start(ur[:], um_d[0:1, sl])
                pu = pst()
                nc.tensor.matmul(pu[:], ones1[:], ur[:], start=True, stop=True)
                ub = persist.tile([128, 512], bf16, tag="Usb", bufs=NSL)
                nc.any.tensor_copy(ub[:], pu[:])
                U_sb.append(ub)
                mr = work.tile([1, 512], bf16, tag="umrow", bufs=2)
                nc.sync.dma_start(mr[:], um_d[1:2, sl])
                pm = pst()
                nc.tensor.matmul(pm[:], ones1[:], mr[:], start=True, stop=True)
                mb = persist.tile([128, 512], bf16, tag="Msb", bufs=NSL)
                nc.any.tensor_copy(mb[:], pm[:])
                M_sb.append(mb)

            # ---------------- LayerNorm + FiLM apply (in place) ----------------
            for kt in range(KT):
                for isl in range(NSL):
                    b = isl // (NSL // B)
                    sl = slice(isl * 512, (isl + 1) * 512)
                    t1 = work.tile([128, 512], bf16, tag="t1", bufs=4)
                    nc.vector.tensor_tensor(t1[:], x_bf[kt][:, sl], M_sb[isl][:], op=ALU.subtract)
                    t2 = work.tile([128, 512], bf16, tag="t2", bufs=4)
                    nc.vector.tensor_tensor(t2[:], t1[:], U_sb[isl][:], op=ALU.mult)
                    col = b * KT + kt
                    nc.vector.tensor_scalar(
                        x_bf[kt][:, sl], t2[:],
                        gpf[:, col:col + 1], bp[:, col:col + 1], ALU.mult, ALU.add,
                    )

            # ---------------- QKV projections (v transposed to natural on the fly) ----------------
            qkvT = []
            for p in range(2):
                qt = persist.tile([128, TOK], bf16, tag="qkvT", bufs=2)
                for isl in range(NSL):
                    sl = slice(isl * 512, (isl + 1) * 512)
                    pq = pst()
                    for kt in range(KT):
                        nc.tensor.matmul(pq[:], wqkv_bf[kt][:, p * 128:(p + 1) * 128],
                                         x_bf[kt][:, sl],
                                         start=(kt == 0), stop=(kt == KT - 1))
                    nc.any.tensor_copy(qt[:, sl], pq[:])
                qkvT.append(qt)
            q2T, k2T = qkvT

            V2 = [None] * (B * JT)
            for isl in range(NSL):
                sl = slice(isl * 512, (isl + 1) * 512)
                pq = pst()
                for kt in range(KT):
                    nc.tensor.matmul(pq[:], wqkv_bf[kt][:, 256:384], x_bf[kt][:, sl],
                                     start=(kt == 0), stop=(kt == KT - 1))
                vtile = work.tile([128, 512], bf16, tag="vtile", bufs=3)
                nc.any.tensor_copy(vtile[:], pq[:])
                for q4 in range(4):
                    jt = isl * 4 + q4
                    pv = pst((128, 128), bf16)
                    nc.tensor.transpose(pv[:], vtile[:, q4 * 128:(q4 + 1) * 128], ident[:])
                    v2 = persist.tile([128, 128], bf16, tag="V2", bufs=B * JT)
                    nc.any.tensor_copy(v2[:], pv[:])
                    V2[jt] = v2

            # ---------------- attention + output projection, per batch ----------------
            o2t = persist.tile([128, TOK], bf16, tag="o2t")
            for b in range(B):
                bo = b * N
                osb = []
                for isl in range(4):
                    po_h0 = pst()
                    po_h1 = pst()
                    pd0 = pst()
                    pd1 = pst()
                    qsl = slice(bo + isl * 512, bo + (isl + 1) * 512)
                    for jt in range(JT):
                        ksl = slice(bo + jt * 128, bo + (jt + 1) * 128)
                        st0 = pst()
                        st1 = pst()
                        nc.tensor.matmul(st0[:], k2T[0:64, ksl], q2T[0:64, qsl],
                                         start=True, stop=True)
                        nc.tensor.matmul(st1[:], k2T[64:128, ksl], q2T[64:128, qsl],
                                         start=True, stop=True)
                        pt0 = work.tile([128, 512], bf16, tag="pt", bufs=4)
                        nc.scalar.activation(pt0[:], st0[:], AF.Exp, scale=DH ** -0.5)
                        pt1 = work.tile([128, 512], bf16, tag="pt", bufs=4)
                        nc.scalar.activation(pt1[:], st1[:], AF.Exp, scale=DH ** -0.5)
                        gj = b * JT + jt
                        nc.tensor.matmul(po_h0[0:64, :], V2[gj][:, 0:64], pt0[:],
                                         start=(jt == 0), stop=(jt == JT - 1))
                        nc.tensor.matmul(po_h1[64:128, :], V2[gj][:, 64:128], pt1[:],
                                         start=(jt == 0), stop=(jt == JT - 1))
                        nc.tensor.matmul(pd0[0:1, :], ones_col[:], pt0[:],
                                         start=(jt == 0), stop=(jt == JT - 1))
                        nc.tensor.matmul(pd1[32:33, :], ones_col[:], pt1[:],
                                         start=(jt == 0), stop=(jt == JT - 1))
                    ob = work.tile([128, 512], f32, tag="osb", bufs=4)
                    nc.any.tensor_copy(ob[0:64, :], po_h0[0:64, :])
                    nc.any.tensor_copy(ob[64:128, :], po_h1[64:128, :])
                    osb.append(ob)
                    dstage = work.tile([128, 512], f32, tag="dstage", bufs=2)
                    nc.any.tensor_copy(dstage[0:1, :], pd0[0:1, :])
                    nc.any.tensor_copy(dstage[32:33, :], pd1[32:33, :])
                    nc.sync.dma_start(den_d[b, isl, 0], dstage[0:1, :])
                    nc.sync.dma_start(den_d[b, isl, 1], dstage[32:33, :])
                # reciprocal over the batch's 8 denominator rows
                denp = work.tile([8, 512], f32, tag="denp", bufs=1)
                nc.sync.dma_start(denp[:], den_d[b].rearrange("i h x -> (i h) x"))
                rp = work.tile([8, 512], f32, tag="rp", bufs=1)
                nc.vector.reciprocal(rp[:], denp[:])
                rpb = work.tile([8, 512], bf16, tag="rpb", bufs=1)
                nc.vector.tensor_copy(rpb[:], rp[:])
                nc.sync.dma_start(r_d[b].rearrange("i h x -> (i h) x"), rpb[:])

                # normalize: O2t = O * R2 (broadcast reciprocal)
                for isl in range(4):
                    sl = slice(isl * 512, (isl + 1) * 512)
                    rp_isl = work.tile([2, 512], bf16, tag="rpisl", bufs=2)
                    nc.sync.dma_start(rp_isl[:], r_d[b].rearrange("i h x -> h i x")[:, isl:isl + 1])
                    pr = pst()
                    nc.tensor.matmul(pr[:], ones2[:], rp_isl[:], start=True, stop=True)
                    r2 = work.tile([128, 512], f32, tag="r2sb", bufs=2)
                    nc.any.tensor_copy(r2[:], pr[:])
                    osl = slice(bo + isl * 512, bo + (isl + 1) * 512)
                    nc.vector.tensor_tensor(o2t[0:64, osl], osb[isl][0:64, :], r2[0:64, :], op=ALU.mult)
                    nc.vector.tensor_tensor(o2t[64:128, osl], osb[isl][64:128, :], r2[64:128, :], op=ALU.mult)
                # output projection for this batch
                for ncx in range(8):
                    for ts in range(4):
                        sl = slice(bo + ts * 512, bo + (ts + 1) * 512)
                        py = pst()
                        nc.tensor.matmul(py[:], wo_bf[:, ncx * 128:(ncx + 1) * 128],
                                         o2t[:, sl], start=True, stop=True)
                        yb = work.tile([128, 512], bf16, tag="ysb", bufs=3)
                        nc.any.tensor_copy(yb[:], py[:])
                        nc.sync.dma_start(yT_out[ncx * 128:(ncx + 1) * 128, sl], yb[:])

    nc.compile()
    return nc


_NC_CACHE = None


def _get_nc():
    global _NC_CACHE
    if _NC_CACHE is None:
        _NC_CACHE = build_program()
    return _NC_CACHE


def make_in_maps(x, conditioning_embeddings, gamma, cond_W, cond_b, Wq, Wkv, Wo):
    x = np.asarray(x, np.float32)
    ce = np.asarray(conditioning_embeddings, np.float32)
    gamma = np.asarray(gamma, np.float32)
    cond_W = np.asarray(cond_W, np.float32)
    cond_b = np.asarray(cond_b, np.float32)
    Wq = np.asarray(Wq, np.float32)
    Wkv = np.asarray(Wkv, np.float32)
    Wo = np.asarray(Wo, np.float32)

    xT = np.ascontiguousarray(x.reshape(TOK, DIM).T)
    ceT = np.ascontiguousarray(ce.reshape(B, KT, 128).transpose(2, 1, 0).reshape(128, 2 * KT))
    gammaT = np.ascontiguousarray(gamma.reshape(KT, 128).T)
    condb2 = np.ascontiguousarray(np.broadcast_to(cond_b, (2, 2 * DIM)))
    ones2 = np.zeros((2, 128), np.float32)
    ones2[0, 0:64] = 1.0
    ones2[1, 64:128] = 1.0
    ones2 = ones2.astype(ml_dtypes.bfloat16)

    in_maps = []
    for c in range(NCORES):
        cs = slice(128 * c, 128 * (c + 1))
        wqkv = np.ascontiguousarray(
            np.concatenate([Wq[:, cs], Wkv[:, cs], Wkv[:, 1024 + 128 * c:1024 + 128 * (c + 1)]], axis=1)
        )
        in_maps.append({
            "xT": xT,
            "ceT": ceT,
            "gammaT": gammaT,
            "condW": cond_W,
            "condb": condb2,
            "wqkv": wqkv,
            "wo": np.ascontiguousarray(Wo[cs, :]),
            "ones2": ones2,
        })
    return in_maps


def kernel(**inputs) -> np.ndarray:
    nc = _get_nc()
    in_maps = make_in_maps(**inputs)
    res = run_bass_kernel_spmd(nc, in_maps, core_ids=list(range(NCORES)))
    acc = np.zeros((DIM, TOK), np.float32)
    for core in res.results:
        acc += np.asarray(core["yT"]).astype(np.float32)
    return np.ascontiguousarray(acc.T).reshape(B, N, DIM)
